# revision 2
# baseline (speedup 1.0000x reference)
"""Expert-parallel MoE kernel for Trainium2 (8 NeuronCores) — v4.

Sharding: core e owns expert e (host routes tokens; device computes all
output numerics: gate recomputed on device, expert MLP, weighted sum).

v4 changes vs v3 (v3 measured 1032 us HW vs 515 us CoreSim => PE idle):
  - W1 AND W2 both SBUF-resident in fp16 (64 KB/partition each), loaded
    once outside the rep loop.  v3 re-streamed all of W1 (8 MiB) per
    512-token block => 40 MiB/rep of HBM traffic whose latency the PE
    ended up waiting on.  v4 steady-state DMA is x in (1 MiB/block) and
    outT out (1 MiB/block fp16) only.
  - gate transpose/broadcast moved AFTER layer-1 matmul issue so the PE
    queue never waits on the gate's DVE chain (v3 stalled PE at the
    transposes between gate matmuls and layer-1 matmuls every block).
  - outT written as fp16 (host accumulates in fp32; adds ~5e-4 rel err
    against a 2e-2 budget) in one DMA per block instead of 8.
"""

import math
import os
import sys

import numpy as np

sys.path.insert(0, "/opt/trn_rl_repo")

P = 128
E = 8
DIN = 1024
DH = 4096
DO = 1024
KC = DIN // P   # 8  k-chunks of x / W1 contraction
HC = DH // P    # 32 h-chunks of W2 contraction
HCG = 8         # W1 grouped as 8 x 512 h-columns
DOC = DO // P   # 8  output chunks
NCORES = 8
TBMAX = 512     # tokens per block
NSBMAX = TBMAX // P
BIG = 1.0e30

_compiled = {}
_wcache = {}
LAST_DISPATCH_S = None
LAST_RES = None


def _build(blocks, reps):
    import concourse.mybir as mybir
    import concourse.tile as tile
    from concourse import bacc

    F32 = mybir.dt.float32
    F16 = mybir.dt.float16
    X = mybir.AxisListType.X

    nc = bacc.Bacc("TRN2", target_bir_lowering=False, debug=False,
                   num_devices=NCORES)

    cap = sum(blocks)
    S = cap // P

    xT = nc.dram_tensor("xT", [P, KC, cap], F16, kind="ExternalInput").ap()
    W1g = nc.dram_tensor("W1g", [P, HCG, KC, 4 * P], F16,
                         kind="ExternalInput").ap()
    W2d = nc.dram_tensor("W2d", [P, DOC, HC, P], F16,
                         kind="ExternalInput").ap()
    Wgm = nc.dram_tensor("Wgm", [P, KC, E], F16, kind="ExternalInput").ap()
    b1c = nc.dram_tensor("b1c", [P, HC], F32, kind="ExternalInput").ap()
    b2t = nc.dram_tensor("b2t", [P, DOC], F32, kind="ExternalInput").ap()
    bgr = nc.dram_tensor("bgr", [P, NSBMAX, E], F32,
                         kind="ExternalInput").ap()
    sel4 = nc.dram_tensor("sel4", [P, NSBMAX, E], F32,
                          kind="ExternalInput").ap()
    idm = nc.dram_tensor("idm", [P, P], F32, kind="ExternalInput").ap()
    # transposed output: outT[p, doc, t] = out[t, doc*128+p]
    outT = nc.dram_tensor("outT", [P, DOC, cap], F16, kind="ExternalOutput").ap()

    with tile.TileContext(nc) as tc:
        with tc.tile_pool(name="const", bufs=1) as cpool, \
             tc.tile_pool(name="xtp", bufs=2) as xtp, \
             tc.tile_pool(name="htp", bufs=1) as htp, \
             tc.tile_pool(name="obp", bufs=1) as obp, \
             tc.tile_pool(name="wrp", bufs=2) as wrp, \
             tc.tile_pool(name="gate", bufs=1) as gpool, \
             tc.tile_pool(name="ps", bufs=6, space="PSUM") as ps, \
             tc.tile_pool(name="psw", bufs=1, space="PSUM") as psw, \
             tc.tile_pool(name="psg", bufs=1, space="PSUM") as psg:

            # SBUF-resident weights + constants, loaded once.  Weights go on
            # the scalar DMA ring in per-group chunks so the sync ring (xt,
            # outT) isn't stuck behind 16 MiB on first dispatch, and layer-1
            # group 0 can start as soon as its 1 MiB arrives.
            w1_sb = cpool.tile([P, HCG, KC, 4 * P], F16)
            for hcg in range(HCG):
                nc.scalar.dma_start(w1_sb[:, hcg], W1g[:, hcg])
            w2_sb = cpool.tile([P, DOC, HC, P], F16)
            for doc in range(DOC):
                nc.scalar.dma_start(w2_sb[:, doc], W2d[:, doc])
            wg_sb = cpool.tile([P, KC, E], F16)
            nc.sync.dma_start(wg_sb[:], Wgm[:])
            bg_sb = cpool.tile([P, NSBMAX, E], F32)
            nc.sync.dma_start(bg_sb[:], bgr[:])
            b1_sb = cpool.tile([P, HC], F32)
            nc.sync.dma_start(b1_sb[:], b1c[:])
            b2_sb = cpool.tile([P, DOC], F32)
            nc.sync.dma_start(b2_sb[:], b2t[:])
            sel_sb = cpool.tile([P, NSBMAX, E], F32)
            nc.sync.dma_start(sel_sb[:], sel4[:])
            id_sb = cpool.tile([P, P], F32)
            nc.sync.dma_start(id_sb[:], idm[:])
            wcol_all = cpool.tile([P, S, 1], F32)

            def gate_block(xt, tb, s0):
                """This core's per-token gate weight for one block."""
                nsb = tb // P
                lgb = gpool.tile([P, NSBMAX, E], F32, tag="lgb",
                                 name="lgb")[:, :nsb]
                # All nsb sub-chunks accumulate into one PSUM bank as
                # sequential region groups (start= only clears has_written
                # bits; finished regions' values persist), then one DVE add.
                gps = psg.tile([P, NSBMAX, E], F32, tag="gps",
                               name="gps")[:, :nsb]
                for s in range(nsb):
                    for kc in range(KC):
                        nc.tensor.matmul(
                            gps[:, s, :], xt[:, kc, s * P:(s + 1) * P],
                            wg_sb[:, kc, :],
                            start=(kc == 0), stop=(kc == KC - 1))
                nc.vector.tensor_tensor(
                    lgb[:], gps[:], bg_sb[:, :nsb], mybir.AluOpType.add)

                gw = gpool.tile([P, NSBMAX, 28], F32, tag="gw", name="gw")
                _c = [0]

                def g(w):
                    c = _c[0]
                    _c[0] += w
                    return gw[:, :nsb, c:c + w]

                m1 = g(1)
                nc.vector.reduce_max(m1[:], lgb[:], axis=X)
                eq = g(E)
                nc.vector.tensor_tensor(eq[:], lgb[:],
                                        m1.to_broadcast([P, nsb, E]),
                                        mybir.AluOpType.is_ge)
                cnt = g(1)
                nc.vector.reduce_sum(cnt[:], eq[:], axis=X)
                tmp = g(E)
                nc.vector.tensor_scalar_mul(tmp[:], eq[:], BIG)
                nc.vector.tensor_sub(tmp[:], lgb[:], tmp[:])
                m2 = g(1)
                nc.vector.reduce_max(m2[:], tmp[:], axis=X)
                msk = g(1)
                nc.vector.tensor_scalar(msk[:], cnt[:], 2.0, None,
                                        mybir.AluOpType.is_ge)
                dd = g(1)
                nc.vector.tensor_sub(dd[:], m1[:], m2[:])
                nc.vector.tensor_tensor(dd[:], dd[:], msk[:],
                                        mybir.AluOpType.mult)
                nc.vector.tensor_add(m2[:], m2[:], dd[:])
                lsel = g(1)
                wst = gpool.tile([P, NSBMAX, E], F32, tag="wst",
                                 name="wst")[:, :nsb]
                nc.vector.tensor_tensor(wst[:], lgb[:], sel_sb[:, :nsb],
                                        mybir.AluOpType.mult)
                nc.vector.reduce_sum(lsel[:], wst[:], axis=X)
                d2 = g(1)
                nc.vector.tensor_sub(d2[:], m2[:], m1[:])
                e2 = g(1)
                nc.scalar.activation(e2[:], d2[:],
                                     mybir.ActivationFunctionType.Exp)
                den = g(1)
                nc.vector.tensor_scalar_add(den[:], e2[:], 1.0)
                rec = g(1)
                nc.vector.reciprocal(rec[:], den[:])
                dsel = g(1)
                nc.vector.tensor_sub(dsel[:], lsel[:], m1[:])
                wex = g(1)
                nc.scalar.activation(wex[:], dsel[:],
                                     mybir.ActivationFunctionType.Exp)
                nc.vector.tensor_tensor(wcol_all[:, s0:s0 + nsb], wex[:],
                                        rec[:], mybir.AluOpType.mult)

            def body(_iv=None):
                s0 = 0
                for tb in blocks:
                    nsb = tb // P
                    t0 = s0 * P

                    xt = xtp.tile([P, KC, TBMAX], F16, tag="xt",
                                  name="xt")[:, :, :tb]
                    nc.sync.dma_start(xt[:], xT[:, :, t0:t0 + tb])

                    # gate matmuls + DVE chain (wcol ready well before L2)
                    gate_block(xt, tb, s0)

                    # layer 1: hT[hc] = relu(W1[:, :, hc].T @ x + b1[hc])
                    hT = htp.tile([P, HC, TBMAX], F16, tag="hT",
                                  name="hT")[:, :, :tb]
                    for hcg in range(HCG):
                        for j in range(4):
                            hc = hcg * 4 + j
                            ps1 = ps.tile([P, TBMAX], F32, tag="mm",
                                          name="mm")[:, :tb]
                            for kc in range(KC):
                                nc.tensor.matmul(
                                    ps1[:],
                                    w1_sb[:, hcg, kc, j * P:(j + 1) * P],
                                    xt[:, kc, :],
                                    start=(kc == 0), stop=(kc == KC - 1))
                            nc.scalar.activation(
                                hT[:, hc, :], ps1[:],
                                mybir.ActivationFunctionType.Relu,
                                bias=b1_sb[:, hc:hc + 1], scale=1.0)

                    # w^T broadcast to all partitions — emitted after L1 so
                    # the PE transposes never make the PE queue wait on the
                    # gate's DVE chain.
                    wtp = psw.tile([P, TBMAX], F32, tag="wtp",
                                   name="wtp")[:, :tb]
                    for s in range(nsb):
                        nc.tensor.transpose(
                            wtp[:1, s * P:(s + 1) * P],
                            wcol_all[:, s0 + s, :], id_sb[:])
                    wrow = wrp.tile([1, TBMAX], F32, tag="wrow",
                                    name="wrow")[:, :tb]
                    nc.vector.tensor_copy(wrow[:], wtp[:1, :tb])
                    wrep = wrp.tile([P, TBMAX], F32, tag="wrep",
                                    name="wrep")[:, :tb]
                    nc.gpsimd.partition_broadcast(wrep[:], wrow[:])

                    # layer 2 (transposed): out[doc] = W2[:, doc].T @ hT
                    ob = obp.tile([P, DOC, TBMAX], F16, tag="ob",
                                  name="ob")[:, :, :tb]
                    for doc in range(DOC):
                        ps2 = ps.tile([P, TBMAX], F32, tag="mm",
                                      name="mm")[:, :tb]
                        for hc in range(HC):
                            nc.tensor.matmul(
                                ps2[:], w2_sb[:, doc, hc, :], hT[:, hc, :],
                                start=(hc == 0), stop=(hc == HC - 1))
                        nc.vector.scalar_tensor_tensor(
                            ob[:, doc, :], ps2[:], b2_sb[:, doc:doc + 1],
                            wrep[:],
                            mybir.AluOpType.add, mybir.AluOpType.mult)
                    nc.sync.dma_start(outT[:, :, t0:t0 + tb], ob[:])
                    s0 += nsb

            if reps > 1:
                with tc.For_i(0, reps, 1) as _i:
                    body(_i)
            else:
                body()

    nc.compile()
    return nc


def _get_compiled(blocks, reps):
    key = (tuple(blocks), reps)
    if key not in _compiled:
        _compiled[key] = _build(blocks, reps)
    return _compiled[key]


def _weights_f16(W1, b1, W2, b2, Wg, bg):
    """Per-expert relaid-out fp16 weights (cached across calls)."""
    key = (id(W1), id(W2), id(Wg))
    hit = _wcache.get(key)
    if hit is not None and hit[0] is W1 and hit[1] is W2 and hit[2] is Wg:
        return hit[3]
    Wgm = np.ascontiguousarray(
        Wg.reshape(KC, P, E).transpose(1, 0, 2).astype(np.float16))
    bgrr = np.tile(bg.astype(np.float32), (P, NSBMAX, 1))
    per_e = []
    for e in range(E):
        # W1g[p, hcg, kc, c] = W1[e, kc*128+p, hcg*512+c]
        w1g = np.ascontiguousarray(
            W1[e].reshape(KC, P, HCG, 4 * P).transpose(1, 2, 0, 3)
            .astype(np.float16))
        # W2d[p, doc, hc, c] = W2[e, hc*128+p, doc*128+c]
        w2d = np.ascontiguousarray(
            W2[e].reshape(HC, P, DOC, P).transpose(1, 2, 0, 3)
            .astype(np.float16))
        b1cc = np.ascontiguousarray(b1[e].reshape(HC, P).T.astype(np.float32))
        b2tt = np.ascontiguousarray(b2[e].reshape(DOC, P).T.astype(np.float32))
        per_e.append((w1g, w2d, b1cc, b2tt))
    out = (Wgm, bgrr, per_e)
    _wcache.clear()
    _wcache[key] = (W1, W2, Wg, out)
    return out


def kernel(x, Wg, bg, W1, b1, W2, b2):
    import time as _time

    from concourse.bass_utils import run_bass_kernel_spmd

    x = np.ascontiguousarray(np.asarray(x, dtype=np.float32))
    Wg = np.ascontiguousarray(np.asarray(Wg, dtype=np.float32))
    bg = np.ascontiguousarray(np.asarray(bg, dtype=np.float32))
    W1 = np.ascontiguousarray(np.asarray(W1, dtype=np.float32))
    b1 = np.ascontiguousarray(np.asarray(b1, dtype=np.float32))
    W2 = np.ascontiguousarray(np.asarray(W2, dtype=np.float32))
    b2 = np.ascontiguousarray(np.asarray(b2, dtype=np.float32))

    T = x.shape[0]

    # Host-side routing (float64) decides the shards only.
    logits = x.astype(np.float64) @ Wg.astype(np.float64) + bg.astype(np.float64)
    top2 = np.argpartition(logits, -2, axis=1)[:, -2:]
    sel_mask = np.zeros((T, E), dtype=bool)
    sel_mask[np.arange(T)[:, None], top2] = True

    idx_e = [np.nonzero(sel_mask[:, e])[0] for e in range(E)]
    counts = [len(i) for i in idx_e]
    cap = max(P, int(math.ceil(max(counts) / P)) * P)
    # Balanced blocks (all >=384 where possible): a 128-token tail block
    # would run 512 weight-load-bound N=128 matmuls; spreading tokens
    # evenly keeps every matmul's free dim large enough to hide LDWEIGHTS.
    nb = max(1, int(math.ceil(cap / TBMAX)))
    sub = cap // P
    base, extra = divmod(sub, nb)
    blocks = [(base + (1 if i < extra else 0)) * P for i in range(nb)]

    reps = int(os.environ.get("MOE_REPS", "1"))
    nc = _get_compiled(blocks, reps)

    Wgm, bgrr, per_e = _weights_f16(W1, b1, W2, b2, Wg, bg)
    x16 = x.astype(np.float16)
    idmat = np.eye(P, dtype=np.float32)

    in_maps = []
    for e in range(E):
        n = counts[e]
        xe = np.zeros((cap, DIN), dtype=np.float16)
        xe[:n] = x16[idx_e[e]]
        sel = np.zeros(E, dtype=np.float32)
        sel[e] = 1.0
        w1g, w2d, b1cc, b2tt = per_e[e]
        in_maps.append({
            "xT": np.ascontiguousarray(
                xe.T.reshape(KC, P, cap).transpose(1, 0, 2)),
            "W1g": w1g,
            "W2d": w2d,
            "Wgm": Wgm,
            "b1c": b1cc,
            "b2t": b2tt,
            "bgr": bgrr,
            "sel4": np.tile(sel, (P, NSBMAX, 1)),
            "idm": idmat,
        })

    _t0 = _time.time()
    res = run_bass_kernel_spmd(nc, in_maps, list(range(NCORES)))
    global LAST_DISPATCH_S, LAST_RES
    LAST_DISPATCH_S = _time.time() - _t0
    LAST_RES = res

    outf = np.zeros((T, DO), dtype=np.float32)
    for e in range(E):
        oT = res.results[e]["outT"]                  # [P, DOC, cap] fp16
        # out[t, doc*128+p] = outT[p, doc, t]
        oe = oT.transpose(2, 1, 0).reshape(cap, DO).astype(np.float32)
        outf[idx_e[e]] += oe[:counts[e]]
    return outf
